# revision 8
# baseline (speedup 1.0000x reference)
"""NT-Xent (SimCLR) contrastive loss on 8 Trainium2 NeuronCores — v2.

Strategy (fully SPMD, no collectives, no on-device transposes):
  Host supplies emb already FEATURE-MAJOR: emb[p, kc, c] = z_row[c][kc*128+p]
  (concat(emb_i, emb_j) rotated by -core*1024 rows, then transposed — a
  layout-only transform). Each core computes its first-1024-column block of
  sim against all 8192 columns.

  Per core, streamed over 8 column-chunks of 1024:
    - Pool: sq = emb_chunk^2 (bf16)
    - PE: ssq[c] = ones[128,128].T @ sq  (partition reduction, replicated
      across all 128 partitions of the psum output)
    - ACT: rinv_s = exp(-0.5*ln(ssq) + ln(16))   [= 16/||e_c||]
    - DVE: zT8 = fp8_e4m3(emb_chunk * rinv_s)    [scaled unit vectors]
    - PE: row-block sim via fp8 DoubleRow matmuls (256 k per instr,
      0.5 cycles/row): q = (16 z)·(16 z) = 256*sim
    - ACT: exp(q/128) with free-dim accumulation -> per-row denominators
      (exp matrix never stored)
  Self-similarity term is the constant e^2 (rows are unit norm).
  Positive-pair dots from a bf16 copy of columns [0:1024) and [4096:5120)
  via DVE multiply+accumulate (partition partials; summed on host).
  Outputs per core: lnd [128, 8] = ln(denom) per row; pdo [128, 4] =
  partition-partial sums of 256*posdot. Host: mean over all rows.
"""

import math

import numpy as np

import concourse.bacc as bacc
import concourse.tile as tile
from concourse import mybir
from concourse.bass_utils import run_bass_kernel_spmd

N_CORES = 8
D = 512
ROWS = 8192
BLK = ROWS // N_CORES  # 1024
P = 128
KD = D // P  # 4 feature chunks of 128
CH = 1024  # column chunk
N_CH = ROWS // CH  # 8
M_CH = BLK // P  # 8 row-block tiles
S = 16.0  # fp8 scale: z stored as 16*z
ESQ = float(np.exp(2.0))  # self-similarity term exp(2*1)

f32 = mybir.dt.float32
bf16 = mybir.dt.bfloat16
fp8 = mybir.dt.float8e4

_ACT_PATCHED = False


def _patch_act_tables():
    """Make Exp and Ln resolve only to natural_log_exp_and_others so the
    whole kernel uses a single activation-table set (no table swaps)."""
    global _ACT_PATCHED
    if _ACT_PATCHED:
        return
    import concourse.hw_specs as hw_specs

    Act = mybir.ActivationFunctionType
    orig = hw_specs.get_activation_tables("gen3")
    patched = {}
    for name, funcs in orig.items():
        fs = set(funcs)
        if name != "natural_log_exp_and_others":
            fs.discard(Act.Exp)
            fs.discard(Act.Ln)
        patched[name] = fs
    bacc.get_activation_tables = lambda arch: patched
    _ACT_PATCHED = True


def _build():
    Alu = mybir.AluOpType
    Act = mybir.ActivationFunctionType
    DR = mybir.MatmulPerfMode.DoubleRow

    _patch_act_tables()
    nc = bacc.Bacc("TRN2", target_bir_lowering=False)
    emb = nc.dram_tensor("emb", [P, KD, ROWS], f32, kind="ExternalInput")
    lnd = nc.dram_tensor("lnd", [P, M_CH], f32, kind="ExternalOutput")
    pdo = nc.dram_tensor("pdo", [P, KD], f32, kind="ExternalOutput")

    with tile.TileContext(nc) as tc:
        with (
            tc.tile_pool(name="persist", bufs=1) as persist,
            tc.tile_pool(name="loads", bufs=3) as loads,
            tc.tile_pool(name="sqs", bufs=2) as sqs,
            tc.tile_pool(name="rvs", bufs=2) as rvs,
            tc.tile_pool(name="small", bufs=2) as small,
            tc.tile_pool(name="exs", bufs=2) as exs,
            tc.tile_pool(name="pssq", bufs=1, space="PSUM") as pssq,
            tc.tile_pool(name="psbig", bufs=3, space="PSUM") as psbig,
        ):
            # persistent tensors
            ones = persist.tile([P, P], bf16, tag="ones", name="ones")
            nc.vector.memset(ones, 1.0)
            zT8 = [
                persist.tile([P, 2, ROWS], fp8, tag=f"zT8_{h}", name=f"zT8_{h}")
                for h in range(2)
            ]
            zbf = persist.tile([P, KD, 2 * BLK], bf16, tag="zbf", name="zbf")
            acc = persist.tile([P, M_CH, N_CH], f32, tag="acc", name="acc")

            def normalize(c0, W, et_tag):
                """DMA cols [c0, c0+W), square, partition-reduce, rsqrt,
                convert into zT8 (and zbf for the pos-pair column ranges)."""
                et = loads.tile([P, KD, W], f32, tag=et_tag)
                for kp in range(2):
                    nc.sync.dma_start(
                        out=et[:, 2 * kp : 2 * kp + 2, :],
                        in_=emb[:, 2 * kp : 2 * kp + 2, c0 : c0 + W],
                    )
                sq = sqs.tile([P, KD, W], bf16, tag=f"sq{et_tag}")
                for kp in range(2):
                    nc.gpsimd.tensor_tensor(
                        out=sq[:, 2 * kp : 2 * kp + 2, :],
                        in0=et[:, 2 * kp : 2 * kp + 2, :],
                        in1=et[:, 2 * kp : 2 * kp + 2, :],
                        op=Alu.mult,
                    )
                rv = rvs.tile([P, W], f32, tag=f"rv{et_tag}")
                nsl = (W + 511) // 512
                wsl = W // nsl
                ps_s = pssq.tile([P, 2, 512], f32, tag="pssq")
                for sl in range(nsl):
                    for kc in range(KD):
                        nc.tensor.matmul(
                            ps_s[:, sl, :wsl],
                            ones[:, :],
                            sq[:, kc, sl * wsl : (sl + 1) * wsl],
                            start=(kc == 0),
                            stop=(kc == KD - 1),
                        )
                lnt = small.tile([P, W], f32, tag=f"lnt{et_tag}")
                # rinv_s = exp(-0.5*ln(ssq/S^2)) = S/sqrt(ssq)
                nc.scalar.activation(
                    out=lnt, in_=ps_s[:, 0:nsl, 0:wsl], func=Act.Ln, scale=1.0 / (S * S)
                )
                nc.scalar.activation(out=rv, in_=lnt, func=Act.Exp, scale=-0.5)
                for kc in range(KD):
                    h, j = divmod(kc, 2)
                    nc.vector.scalar_tensor_tensor(
                        out=zT8[h][:, j, c0 : c0 + W],
                        in0=et[:, kc, :],
                        scalar=1.0,
                        in1=rv,
                        op0=Alu.mult,
                        op1=Alu.mult,
                    )
                if c0 < BLK or BLK * 4 <= c0 < BLK * 5:
                    off = c0 if c0 < BLK else c0 - BLK * 4 + BLK
                    for kc in range(KD):
                        nc.vector.scalar_tensor_tensor(
                            out=zbf[:, kc, off : off + W],
                            in0=et[:, kc, :],
                            scalar=1.0,
                            in1=rv,
                            op0=Alu.mult,
                            op1=Alu.mult,
                        )

            # software pipeline: normalize chunk t+1 ahead of chunk t's sim
            # loop so ACT's rsqrt (and DVE's convert) never queue behind the
            # 8 exp instructions of the previous chunk.
            for u in range(4):
                # first chunk at fine granularity: shortens the critical
                # path to the first sim matmul (stationaries live here)
                normalize(u * 256, 256, "eta")
            for t in range(N_CH):
                c0 = t * CH
                if t + 1 < N_CH:
                    normalize((t + 1) * CH, CH, "etb")
                # row-block sim for this column chunk
                for m in range(M_CH):
                    pst = psbig.tile([P, 2, 512], f32, tag="psbig")
                    for h in range(2):
                        for li in range(2):
                            nc.tensor.matmul(
                                pst[:, li, :],
                                zT8[h][:, :, m * P : (m + 1) * P],
                                zT8[h][:, :, c0 + li * 512 : c0 + (li + 1) * 512],
                                start=(h == 0),
                                stop=(h == 1),
                                perf_mode=DR,
                            )
                    ex = exs.tile([P, 2, 512], bf16, tag="ex")
                    nc.scalar.activation(
                        out=ex,
                        in_=pst,
                        func=Act.Exp,
                        scale=2.0 / (S * S),
                        accum_out=acc[:, m, t : t + 1],
                    )

            # positive-pair dots: pdo[p, kc] = sum_c zbf[p,kc,c]*zbf[p,kc,c+BLK]
            pdp = persist.tile([P, KD], f32, tag="pdp", name="pdp")
            for kc in range(KD):
                junk = small.tile([P, BLK], bf16, tag="junk")
                nc.vector.scalar_tensor_tensor(
                    out=junk,
                    in0=zbf[:, kc, 0:BLK],
                    scalar=1.0,
                    in1=zbf[:, kc, BLK : 2 * BLK],
                    op0=Alu.mult,
                    op1=Alu.mult,
                    accum_out=pdp[:, kc : kc + 1],
                )
            # denominators: sum over chunks, subtract self term, ln
            dsum = persist.tile([P, M_CH], f32, tag="dsum", name="dsum")
            for m in range(M_CH):
                nc.vector.reduce_sum(
                    out=dsum[:, m : m + 1], in_=acc[:, m, :], axis=mybir.AxisListType.X
                )
            d2 = persist.tile([P, M_CH], f32, tag="d2", name="d2")
            nc.vector.tensor_scalar_add(out=d2, in0=dsum, scalar1=-ESQ)
            lnt2 = persist.tile([P, M_CH], f32, tag="lnt2", name="lnt2")
            nc.scalar.activation(out=lnt2, in_=d2, func=Act.Ln)
            nc.sync.dma_start(out=lnd[:, :], in_=lnt2)
            nc.sync.dma_start(out=pdo[:, :], in_=pdp)

    nc.compile()
    return nc


_NC_CACHE = []


def _get_nc():
    if not _NC_CACHE:
        _NC_CACHE.append(_build())
    return _NC_CACHE[0]


def make_in_maps(emb_i: np.ndarray, emb_j: np.ndarray):
    emb_all = np.concatenate(
        [np.asarray(emb_i, np.float32), np.asarray(emb_j, np.float32)], axis=0
    )
    maps = []
    for c in range(N_CORES):
        rot = np.roll(emb_all, -c * BLK, axis=0)
        # [p, kc, col] = rot[col, kc*128 + p]
        arr = np.ascontiguousarray(rot.T.reshape(KD, P, ROWS).transpose(1, 0, 2))
        maps.append({"emb": arr})
    return maps


def assemble(results) -> np.ndarray:
    total = 0.0
    for c in range(N_CORES):
        lnd = results[c]["lnd"].astype(np.float64)
        pdo = results[c]["pdo"].astype(np.float64)
        total += lnd.sum() - 2.0 * pdo.sum() / (S * S)
    return np.float32(total / ROWS)


def kernel(emb_i: np.ndarray, emb_j: np.ndarray) -> np.ndarray:
    nc = _get_nc()
    res = run_bass_kernel_spmd(
        nc, make_in_maps(emb_i, emb_j), core_ids=list(range(N_CORES))
    )
    return assemble(res.results)


if __name__ == "__main__":
    rng = np.random.default_rng(0)
    ei = rng.standard_normal((4096, D)).astype(np.float32)
    ej = rng.standard_normal((4096, D)).astype(np.float32)
    print(kernel(ei, ej))


# revision 9
# speedup vs baseline: 1.0277x; 1.0277x over previous
"""NT-Xent (SimCLR) contrastive loss on 8 Trainium2 NeuronCores — v2.

Strategy (fully SPMD, no collectives, no on-device transposes):
  Host supplies emb already FEATURE-MAJOR: emb[p, kc, c] = z_row[c][kc*128+p]
  (concat(emb_i, emb_j) rotated by -core*1024 rows, then transposed — a
  layout-only transform). Each core computes its first-1024-column block of
  sim against all 8192 columns.

  Per core, streamed over 8 column-chunks of 1024:
    - Pool: sq = emb_chunk^2 (bf16)
    - PE: ssq[c] = ones[128,128].T @ sq  (partition reduction, replicated
      across all 128 partitions of the psum output)
    - ACT: rinv_s = exp(-0.5*ln(ssq) + ln(16))   [= 16/||e_c||]
    - DVE: zT8 = fp8_e4m3(emb_chunk * rinv_s)    [scaled unit vectors]
    - PE: row-block sim via fp8 DoubleRow matmuls (256 k per instr,
      0.5 cycles/row): q = (16 z)·(16 z) = 256*sim
    - ACT: exp(q/128) with free-dim accumulation -> per-row denominators
      (exp matrix never stored)
  Self-similarity term is the constant e^2 (rows are unit norm).
  Positive-pair dots from a bf16 copy of columns [0:1024) and [4096:5120)
  via DVE multiply+accumulate (partition partials; summed on host).
  Outputs per core: lnd [128, 8] = ln(denom) per row; pdo [128, 4] =
  partition-partial sums of 256*posdot. Host: mean over all rows.
"""

import math

import numpy as np

import concourse.bacc as bacc
import concourse.tile as tile
from concourse import mybir
from concourse.bass_utils import run_bass_kernel_spmd

N_CORES = 8
D = 512
ROWS = 8192
BLK = ROWS // N_CORES  # 1024
P = 128
KD = D // P  # 4 feature chunks of 128
CH = 1024  # column chunk
N_CH = ROWS // CH  # 8
M_CH = BLK // P  # 8 row-block tiles
S = 16.0  # fp8 scale: z stored as 16*z
ESQ = float(np.exp(2.0))  # self-similarity term exp(2*1)

f32 = mybir.dt.float32
bf16 = mybir.dt.bfloat16
fp8 = mybir.dt.float8e4

_ACT_PATCHED = False


def _patch_act_tables():
    """Make Exp and Ln resolve only to natural_log_exp_and_others so the
    whole kernel uses a single activation-table set (no table swaps)."""
    global _ACT_PATCHED
    if _ACT_PATCHED:
        return
    import concourse.hw_specs as hw_specs

    Act = mybir.ActivationFunctionType
    orig = hw_specs.get_activation_tables("gen3")
    patched = {}
    for name, funcs in orig.items():
        fs = set(funcs)
        if name != "natural_log_exp_and_others":
            fs.discard(Act.Exp)
            fs.discard(Act.Ln)
        patched[name] = fs
    bacc.get_activation_tables = lambda arch: patched
    _ACT_PATCHED = True


def _build():
    Alu = mybir.AluOpType
    Act = mybir.ActivationFunctionType
    DR = mybir.MatmulPerfMode.DoubleRow

    _patch_act_tables()
    nc = bacc.Bacc("TRN2", target_bir_lowering=False)
    emb = nc.dram_tensor("emb", [P, KD, ROWS], f32, kind="ExternalInput")
    lnd = nc.dram_tensor("lnd", [P, M_CH], f32, kind="ExternalOutput")
    pdo = nc.dram_tensor("pdo", [P, 2], f32, kind="ExternalOutput")

    with tile.TileContext(nc) as tc:
        with (
            tc.tile_pool(name="persist", bufs=1) as persist,
            tc.tile_pool(name="loads", bufs=3) as loads,
            tc.tile_pool(name="sqs", bufs=2) as sqs,
            tc.tile_pool(name="rvs", bufs=2) as rvs,
            tc.tile_pool(name="small", bufs=2) as small,
            tc.tile_pool(name="exs", bufs=2) as exs,
            tc.tile_pool(name="pssq", bufs=1, space="PSUM") as pssq,
            tc.tile_pool(name="psbig", bufs=3, space="PSUM") as psbig,
        ):
            # persistent tensors
            ones = persist.tile([P, P], bf16, tag="ones", name="ones")
            nc.vector.memset(ones, 1.0)
            zT8 = [
                persist.tile([P, 2, ROWS], fp8, tag=f"zT8_{h}", name=f"zT8_{h}")
                for h in range(2)
            ]
            acc = persist.tile([P, M_CH, N_CH], f32, tag="acc", name="acc")

            def normalize(c0, W, et_tag):
                """DMA cols [c0, c0+W), square, partition-reduce, rsqrt,
                convert into zT8 (and zbf for the pos-pair column ranges)."""
                et = loads.tile([P, KD, W], f32, tag=et_tag)
                for kp in range(2):
                    nc.sync.dma_start(
                        out=et[:, 2 * kp : 2 * kp + 2, :],
                        in_=emb[:, 2 * kp : 2 * kp + 2, c0 : c0 + W],
                    )
                sq = sqs.tile([P, KD, W], bf16, tag=f"sq{et_tag}")
                for kp in range(2):
                    nc.gpsimd.tensor_tensor(
                        out=sq[:, 2 * kp : 2 * kp + 2, :],
                        in0=et[:, 2 * kp : 2 * kp + 2, :],
                        in1=et[:, 2 * kp : 2 * kp + 2, :],
                        op=Alu.mult,
                    )
                rv = rvs.tile([P, W], f32, tag=f"rv{et_tag}")
                nsl = (W + 511) // 512
                wsl = W // nsl
                ps_s = pssq.tile([P, 2, 512], f32, tag="pssq")
                for sl in range(nsl):
                    for kc in range(KD):
                        nc.tensor.matmul(
                            ps_s[:, sl, :wsl],
                            ones[:, :],
                            sq[:, kc, sl * wsl : (sl + 1) * wsl],
                            start=(kc == 0),
                            stop=(kc == KD - 1),
                        )
                lnt = small.tile([P, W], f32, tag=f"lnt{et_tag}")
                # rinv_s = exp(-0.5*ln(ssq/S^2)) = S/sqrt(ssq)
                nc.scalar.activation(
                    out=lnt, in_=ps_s[:, 0:nsl, 0:wsl], func=Act.Ln, scale=1.0 / (S * S)
                )
                nc.scalar.activation(out=rv, in_=lnt, func=Act.Exp, scale=-0.5)
                for kc in range(KD):
                    h, j = divmod(kc, 2)
                    nc.vector.scalar_tensor_tensor(
                        out=zT8[h][:, j, c0 : c0 + W],
                        in0=et[:, kc, :],
                        scalar=1.0,
                        in1=rv,
                        op0=Alu.mult,
                        op1=Alu.mult,
                    )

            # software pipeline: normalize chunk t+1 ahead of chunk t's sim
            # loop so ACT's rsqrt (and DVE's convert) never queue behind the
            # 8 exp instructions of the previous chunk.
            for u in range(4):
                # first chunk at fine granularity: shortens the critical
                # path to the first sim matmul (stationaries live here)
                normalize(u * 256, 256, "eta")
            for t in range(N_CH):
                c0 = t * CH
                if t + 1 < N_CH:
                    normalize((t + 1) * CH, CH, "etb")
                # row-block sim for this column chunk
                for m in range(M_CH):
                    pst = psbig.tile([P, 2, 512], f32, tag="psbig")
                    for h in range(2):
                        for li in range(2):
                            nc.tensor.matmul(
                                pst[:, li, :],
                                zT8[h][:, :, m * P : (m + 1) * P],
                                zT8[h][:, :, c0 + li * 512 : c0 + (li + 1) * 512],
                                start=(h == 0),
                                stop=(h == 1),
                                perf_mode=DR,
                            )
                    ex = exs.tile([P, 2, 512], bf16, tag="ex")
                    nc.scalar.activation(
                        out=ex,
                        in_=pst,
                        func=Act.Exp,
                        scale=2.0 / (S * S),
                        accum_out=acc[:, m, t : t + 1],
                    )

            # positive-pair dots from fp8 z: pdp[p, h] partial sums of 256*posdot
            pdp = persist.tile([P, 2], f32, tag="pdp", name="pdp")
            for h in range(2):
                junk = small.tile([P, 2, BLK], bf16, tag="junk")
                nc.vector.scalar_tensor_tensor(
                    out=junk,
                    in0=zT8[h][:, :, 0:BLK],
                    scalar=1.0,
                    in1=zT8[h][:, :, 4 * BLK : 5 * BLK],
                    op0=Alu.mult,
                    op1=Alu.mult,
                    accum_out=pdp[:, h : h + 1],
                )
            # denominators: sum over chunks, subtract self term, ln
            dsum = persist.tile([P, M_CH], f32, tag="dsum", name="dsum")
            for m in range(M_CH):
                nc.vector.reduce_sum(
                    out=dsum[:, m : m + 1], in_=acc[:, m, :], axis=mybir.AxisListType.X
                )
            d2 = persist.tile([P, M_CH], f32, tag="d2", name="d2")
            nc.vector.tensor_scalar_add(out=d2, in0=dsum, scalar1=-ESQ)
            lnt2 = persist.tile([P, M_CH], f32, tag="lnt2", name="lnt2")
            nc.scalar.activation(out=lnt2, in_=d2, func=Act.Ln)
            nc.sync.dma_start(out=lnd[:, :], in_=lnt2)
            nc.sync.dma_start(out=pdo[:, :], in_=pdp)

    nc.compile()
    return nc


_NC_CACHE = []


def _get_nc():
    if not _NC_CACHE:
        _NC_CACHE.append(_build())
    return _NC_CACHE[0]


def make_in_maps(emb_i: np.ndarray, emb_j: np.ndarray):
    emb_all = np.concatenate(
        [np.asarray(emb_i, np.float32), np.asarray(emb_j, np.float32)], axis=0
    )
    maps = []
    for c in range(N_CORES):
        rot = np.roll(emb_all, -c * BLK, axis=0)
        # [p, kc, col] = rot[col, kc*128 + p]
        arr = np.ascontiguousarray(rot.T.reshape(KD, P, ROWS).transpose(1, 0, 2))
        maps.append({"emb": arr})
    return maps


def assemble(results) -> np.ndarray:
    total = 0.0
    for c in range(N_CORES):
        lnd = results[c]["lnd"].astype(np.float64)
        pdo = results[c]["pdo"].astype(np.float64)
        total += lnd.sum() - 2.0 * pdo.sum() / (S * S)
    return np.float32(total / ROWS)


def kernel(emb_i: np.ndarray, emb_j: np.ndarray) -> np.ndarray:
    nc = _get_nc()
    res = run_bass_kernel_spmd(
        nc, make_in_maps(emb_i, emb_j), core_ids=list(range(N_CORES))
    )
    return assemble(res.results)


if __name__ == "__main__":
    rng = np.random.default_rng(0)
    ei = rng.standard_normal((4096, D)).astype(np.float32)
    ej = rng.standard_normal((4096, D)).astype(np.float32)
    print(kernel(ei, ej))
